# revision 25
# baseline (speedup 1.0000x reference)
"""Multi-head causal attention (B=2, T=2048, D=2048, H=16, dk=128) on 8 TRN2 NeuronCores.

Strategy (tensor-parallel over heads, 2 heads/core), bf16 matmul operands:
  - Host prep: all operands pre-permuted into the exact SBUF column layouts
    ([128, ...] partition-major) so every big device DMA is a contiguous 2D
    copy — descriptor issue is trivial and transfers run at full HBM rate.
    Everything cast to bfloat16 (PSUM accumulation stays fp32; rel tolerance
    is 2e-2, bf16 operands land ~6e-3).
  - Per core: QT/KT = PE matmuls producing Q^T/K^T layouts [dk, tokens];
    V natural [tokens, dk] via PE transposes. Wo.T (8 MB bf16) is fully
    SBUF-resident, prefetched during the projection phase.
  - A short burst of identity matmuls at t=0 keeps the PE HAM clock warm
    while the first weight/x transfers land.
  - Attention with TRANSPOSED scores: S^T[kk, q] chunks per kk-tile so the
    exp'd probabilities land directly in P^T layout (no PE transposes of P).
    No max-subtraction (scores are ~N(0,1); exp cannot overflow). Softmax
    denominator via ones[128,128] matmul over P^T partitions (result arrives
    pre-broadcast to all partitions); 1/d via DVE reciprocal_approx_fast read
    straight from PSUM. Diagonal kk-tiles stream only the live q-range
    (s_off) through the score, denominator AND context matmuls, so no zero
    fill of P^T is needed and the PE skips the masked triangle at 128-col
    granularity.
  - ctx^T [dk, tokens] per head -> AllToAll (1 MB/core) so each core gets all
    16 heads' ctx^T for its 512-token slice; cme/cmo gathered on the gpsimd
    queue immediately after each collective. Local Wo projection with
    even/odd head split: evens (ready after AllToAll#0) accumulate during
    the second collective, partials spill to SBUF; odds then accumulate in
    fresh PSUM in four o4-groups, each group's DVE merge + output store
    overlapping the next group's matmuls (spreads the 4 MB output DMA).
  - Host: concatenate the 8 row-slices.
  - Queue placement: xt chunk loads + collectives + cm gathers on gpsimd,
    weight loads + csb/out stores on sync, exps + PSUM spills on scalar.
"""

import math
import numpy as np
import ml_dtypes
from contextlib import ExitStack

import concourse.tile as tile
import concourse.mybir as mybir
from concourse import bacc
from concourse.bass_utils import run_bass_kernel_spmd

B, T, D = 2, 2048, 2048
H, DK = 16, 128
NCORES = 8
HL = H // NCORES            # 2 heads per core
OC = HL * DK                # 256 out dims per core
TT = B * T                  # 4096 flat tokens
TCHUNK = 512
NTC = TT // TCHUNK          # 8 token chunks (proj)
NKT = D // 128              # 16 contraction tiles
NQC = T // TCHUNK           # 4 q-chunks per batch
SCALE = 1.0 / math.sqrt(DK)
F32 = mybir.dt.float32
MDT = mybir.dt.bfloat16
BF16 = ml_dtypes.bfloat16
MASK_VAL = -1e30

_CACHE = {}


def build():
    nc = bacc.Bacc("TRN2", target_bir_lowering=False, debug=False, num_devices=NCORES)

    # host-pre-permuted layouts: partition-major, contiguous per DMA
    xt_d = nc.dram_tensor("xt", [128, NTC * NKT * TCHUNK], MDT, kind="ExternalInput")
    wqt_d = nc.dram_tensor("wqt", [128, NKT * OC], MDT, kind="ExternalInput")
    wkt_d = nc.dram_tensor("wkt", [128, NKT * OC], MDT, kind="ExternalInput")
    wvt_d = nc.dram_tensor("wvt", [128, NKT * OC], MDT, kind="ExternalInput")
    wot_d = nc.dram_tensor("wot", [128, NKT * D], MDT, kind="ExternalInput")
    out_d = nc.dram_tensor("out", [TT // NCORES, D], MDT, kind="ExternalOutput")

    with tile.TileContext(nc) as tc, ExitStack() as ctx:
        psum = ctx.enter_context(tc.tile_pool(name="ps", bufs=8, space="PSUM"))
        dram = ctx.enter_context(tc.tile_pool(name="dram", bufs=1, space="DRAM"))
        persist = ctx.enter_context(tc.tile_pool(name="persist", bufs=1))
        small = ctx.enter_context(tc.tile_pool(name="small", bufs=2))

        # ---- persistent SBUF: QT/KT [128, HL*TT] (cols: h*TT + flat_tok),
        # V [128, 32*256], full Wo.T [128, 16*2048] (cols: c16*D + o)
        QT = persist.tile([128, HL * TT], MDT, name="QTs")
        KT = persist.tile([128, HL * TT], MDT, name="KTs")
        Vs = persist.tile([128, (TT // 128) * OC], MDT, name="Vs")

        # ---- identity (needed by proj-phase V transposes + PE warmup)
        ident = persist.tile([128, 128], MDT, name="ident")
        with tc.tile_pool(name="cstage0", bufs=1) as cstage0:
            ident_f = cstage0.tile([128, 128], F32, name="ident_f")
            from concourse.masks import make_identity
            make_identity(nc, ident_f[:])
            nc.vector.tensor_copy(ident[:], ident_f[:])

        # ---- PE warmup: keep HAM at full clock while the first loads land
        warm = psum.tile([128, 128], F32, tag="mm", name="warm")
        for _ in range(40):
            nc.tensor.matmul(warm[:], ident[:], ident[:], start=True, stop=True)

        # =================== Phase 1: QKV projections ===================
        wope = ctx.enter_context(tc.tile_pool(name="wope", bufs=1))
        with tc.tile_pool(name="proj", bufs=1) as projp, \
             tc.tile_pool(name="xtp", bufs=2) as xtp:
            # lo/hi halves as separate tiles so the first matmuls depend only
            # on the first transfers
            HW2 = 8 * OC
            w_sbs = {}
            # one engine queue per weight tensor: three 0.75 MB transfers run
            # concurrently right after the preamble
            for kh in range(2):
                for wname, w_d, eng in (("q", wqt_d, nc.sync), ("k", wkt_d, nc.scalar),
                                        ("v", wvt_d, nc.sync if kh else nc.scalar)):
                    w_sb = projp.tile([128, HW2], MDT, name=f"w{wname}{kh}_sb")
                    w_sbs[(wname, kh)] = w_sb
                    eng.dma_start(out=w_sb[:], in_=w_d.ap()[:, kh * HW2:(kh + 1) * HW2])

            # full Wo.T prefetch (8 MB bf16) in quarters, interleaved on the
            # gpsimd queue late in proj so it never contends with the weight
            # and first-chunk loads; it streams long before phase 3 needs it.
            wo_sb = None
            WOQ = NKT * D // 4

            XW = NKT * TCHUNK
            for tcx in range(NTC):
                xts = []
                for kh in range(2):
                    xt = xtp.tile([128, XW // 2], MDT, tag=f"xt{kh}", name=f"xt_{tcx}_{kh}")
                    xts.append(xt)
                    nc.gpsimd.dma_start(
                        out=xt[:],
                        in_=xt_d.ap()[:, tcx * XW + kh * (XW // 2): tcx * XW + (kh + 1) * (XW // 2)],
                    )
                if tcx >= 3 and tcx % 2 == 1:
                    if wo_sb is None:
                        wo_sb = wope.tile([128, NKT * D], MDT, name="wo_sb")
                    wq4 = (tcx - 3) // 2
                    nc.gpsimd.dma_start(out=wo_sb[:, wq4 * WOQ:(wq4 + 1) * WOQ],
                                        in_=wot_d.ap()[:, wq4 * WOQ:(wq4 + 1) * WOQ])

                qp = [psum.tile([128, 512], F32, tag="mm", name=f"qp{tcx}_{o}") for o in range(HL)]
                kp = [psum.tile([128, 512], F32, tag="mm", name=f"kp{tcx}_{o}") for o in range(HL)]
                # V computed directly in natural [token, head*dk] layout:
                # stationary = x^T token-subtile, moving = Wv k-slice (N=256).
                # Two token-subtiles share one PSUM bank; only the first gets
                # start=True (bank-wide has_written clear), the second then
                # overwrites-where-clear.
                vn = [psum.tile([128, 512], F32, tag="mm", name=f"vn{tcx}_{i}") for i in range(2)]
                for k in range(NKT):
                    st, sp = (k == 0), (k == NKT - 1)
                    kh, kr = divmod(k, 8)
                    xk = xts[kh][:, kr * TCHUNK:(kr + 1) * TCHUNK]
                    for o in range(HL):
                        for w, ps in (("q", qp), ("k", kp)):
                            nc.tensor.matmul(
                                ps[o][:],
                                w_sbs[(w, kh)][:, kr * OC + o * 128: kr * OC + (o + 1) * 128],
                                xk, start=st, stop=sp)
                    wvk = w_sbs[("v", kh)][:, kr * OC:(kr + 1) * OC]
                    for t4 in range(4):
                        nc.tensor.matmul(
                            vn[t4 // 2][:, (t4 % 2) * 256:(t4 % 2) * 256 + 256],
                            xk[:, t4 * 128:(t4 + 1) * 128],
                            wvk,
                            start=(st and t4 % 2 == 0), stop=sp)
                for o in range(HL):
                    dst = slice(o * TT + tcx * TCHUNK, o * TT + (tcx + 1) * TCHUNK)
                    nc.scalar.copy(QT[:, dst], qp[o][:])
                    nc.scalar.copy(KT[:, dst], kp[o][:])
                for i in range(2):
                    t32 = tcx * 4 + 2 * i
                    nc.vector.tensor_copy(Vs[:, t32 * OC: (t32 + 2) * OC], vn[i][:])

        # ---- last Wo.T quarter (emitted after proj)
        nc.gpsimd.dma_start(out=wo_sb[:, 3 * WOQ:4 * WOQ],
                            in_=wot_d.ap()[:, 3 * WOQ:4 * WOQ])

        # ---- attention constants (emitted after proj so the first xt DMAs lead)
        maskT = persist.tile([128, 128], F32, name="maskT")
        nc.gpsimd.memset(maskT[:], 0.0)
        # keep 0 where q >= kk (predicate -x + y >= 0), else MASK_VAL
        nc.gpsimd.affine_select(
            out=maskT[:], in_=maskT[:], compare_op=mybir.AluOpType.is_ge,
            fill=MASK_VAL, base=0, pattern=[[1, 128]], channel_multiplier=-1,
        )
        onesk = persist.tile([128, 128], MDT, name="onesk")
        with tc.tile_pool(name="cstage", bufs=1) as cstage:
            ones_f = cstage.tile([128, 128], F32, name="ones_f")
            nc.gpsimd.memset(ones_f[:], 1.0)
            nc.vector.tensor_copy(onesk[:], ones_f[:])

        # =================== Phase 2: attention ===================
        a2a_in = [dram.tile([NCORES, 128, TCHUNK], MDT, name=f"a2a_in{h}") for h in range(HL)]
        a2a_out = [dram.tile([NCORES, 128, TCHUNK], MDT, name=f"a2a_out{h}")
                   for h in range(HL)]

        cm = [None, None]

        with tc.tile_pool(name="ptp", bufs=2) as ptp, \
             tc.tile_pool(name="rdp", bufs=3) as rdp:
            for hl in range(HL):
                for J in reversed(range(NQC)):
                    for b in range(B):
                        base = hl * TT + b * T
                        nkk = 4 * J + 4
                        ptiles = []
                        for kk in range(nkk):
                            s_off = max(0, (kk - 4 * J) * 128)
                            npr = 512 - s_off
                            pt = ptp.tile([128, 512], MDT, tag=f"pt{kk}", name=f"p_{hl}{b}{J}_{kk}")
                            ptiles.append((pt, s_off))
                            st = psum.tile([128, 512], F32, tag="mm", name=f"st{hl}{b}{J}_{kk}")
                            nc.tensor.matmul(
                                st[:, :npr],
                                KT[:, base + kk * 128: base + (kk + 1) * 128],
                                QT[:, base + J * 512 + s_off: base + (J + 1) * 512],
                                start=True, stop=True,
                            )
                            if kk >= 4 * J:  # diagonal tile: causal mask
                                nc.vector.tensor_add(st[:, 0:128], st[:, 0:128], maskT[:])
                            nc.scalar.activation(pt[:, s_off:512], st[:, :npr],
                                                 mybir.ActivationFunctionType.Exp, scale=SCALE)
                        # denominator (broadcast to all 128 partitions): d[p, q] = sum_kk P^T
                        dp = psum.tile([128, 512], F32, tag="mm", name=f"dp{hl}{b}{J}")
                        for i, (pt, s_off) in enumerate(ptiles):
                            nc.tensor.matmul(dp[:, s_off:512], onesk[:], pt[:, s_off:512],
                                             start=(i == 0), stop=(i == nkk - 1))
                        rd = rdp.tile([128, 512], F32, tag="rd", name=f"rd_{hl}{b}{J}")
                        nc.vector.reciprocal_approx_fast(rd[:], dp[:])
                        # ctx^T accumulate over kk
                        cp = psum.tile([128, 512], F32, tag="mm", name=f"cp{hl}{b}{J}")
                        for i, (pt, s_off) in enumerate(ptiles):
                            nc.tensor.matmul(
                                cp[:, s_off:512],
                                Vs[:, (b * 16 + i) * OC + hl * 128: (b * 16 + i) * OC + (hl + 1) * 128],
                                pt[:, s_off:512],
                                start=(i == 0), stop=(i == nkk - 1),
                            )
                        csb = rdp.tile([128, 512], MDT, tag="csb", name=f"csb{hl}{b}{J}")
                        nc.vector.tensor_mul(csb[:], cp[:], rd[:])
                        nc.sync.dma_start(out=a2a_in[hl][b * NQC + J], in_=csb[:])
                nc.gpsimd.collective_compute(
                    "AllToAll", mybir.AluOpType.bypass,
                    replica_groups=[list(range(NCORES))],
                    ins=[a2a_in[hl].opt()], outs=[a2a_out[hl].opt()],
                )
                # gather all 16 heads' ctx^T for my 512-token slice right away
                cm[hl] = wope.tile([128, NCORES * TCHUNK], MDT, name=f"cm{hl}")
                nc.gpsimd.dma_start(
                    out=cm[hl][:].rearrange("p (c t) -> p c t", c=NCORES),
                    in_=a2a_out[hl].rearrange("c p t -> p c t"),
                )

        # =================== Phase 3: output projection ===================
        # Evens (= heads from AllToAll#0) accumulate for all o4 groups while
        # AllToAll#1 is in flight, spilling partials to SBUF; odds then run
        # in four o4-groups so each group's merge+store overlaps the next
        # group's matmuls.
        with tc.tile_pool(name="accp", bufs=1) as accp, \
             tc.tile_pool(name="outp", bufs=3) as outp:
            acc = {}
            for pi, (oa, ob) in enumerate(((0, 1), (2, 3))):
                ops = {o4: [psum.tile([128, 512], F32, tag="mm", name=f"ope{o4}_{t}")
                            for t in range(4)]
                       for o4 in (oa, ob)}
                for ci in range(NCORES):
                    i = ci
                    for o4 in (oa, ob):
                        for t4 in range(4):
                            nc.tensor.matmul(
                                ops[o4][t4][:],
                                cm[0][:, i * 512 + t4 * 128: i * 512 + (t4 + 1) * 128],
                                wo_sb[:, 2 * i * D + o4 * 512: 2 * i * D + (o4 + 1) * 512],
                                start=(ci == 0), stop=(ci == NCORES - 1))
                for o4 in (oa, ob):
                    for t4 in range(4):
                        a_ = accp.tile([128, 512], F32, name=f"acc{o4}_{t4}")
                        nc.scalar.copy(a_[:], ops[o4][t4][:])
                        acc[(o4, t4)] = a_
            for o4 in range(4):
                ops = [psum.tile([128, 512], F32, tag="mm", name=f"opo{o4}_{t}")
                       for t in range(4)]
                for ci in range(NCORES):
                    i = ci
                    for t4 in range(4):
                        nc.tensor.matmul(
                            ops[t4][:],
                            cm[1][:, i * 512 + t4 * 128: i * 512 + (t4 + 1) * 128],
                            wo_sb[:, (2 * i + 1) * D + o4 * 512: (2 * i + 1) * D + (o4 + 1) * 512],
                            start=(ci == 0), stop=(ci == NCORES - 1))
                for t4 in range(4):
                    ot = outp.tile([128, 512], MDT, tag="ot", name=f"ot{o4}_{t4}")
                    nc.vector.tensor_add(ot[:], ops[t4][:], acc[(o4, t4)][:])
                    nc.sync.dma_start(
                        out=out_d.ap()[t4 * 128:(t4 + 1) * 128, o4 * 512:(o4 + 1) * 512],
                        in_=ot[:],
                    )

    nc.compile()
    return nc


def get_nc():
    if "nc" not in _CACHE:
        _CACHE["nc"] = build()
    return _CACHE["nc"]


def _perm(wT, inner):
    """[NKT*128, inner] row-major -> [128, NKT*inner] partition-major bf16."""
    return np.ascontiguousarray(
        wT.reshape(NKT, 128, inner).transpose(1, 0, 2).reshape(128, NKT * inner)
    ).astype(BF16)


def make_in_maps(x, wq, wk, wv, wo):
    x = np.asarray(x, dtype=np.float32)
    xT = np.ascontiguousarray(x.reshape(TT, D).T)          # [D, TT]
    # [128, tcx, kt, t] layout: per token-chunk contiguous
    xP = np.ascontiguousarray(
        xT.reshape(NKT, 128, NTC, TCHUNK).transpose(1, 2, 0, 3).reshape(128, -1)
    ).astype(BF16)
    woP = _perm(np.ascontiguousarray(np.asarray(wo, np.float32).T), D)
    in_maps = []
    for i in range(NCORES):
        sl = slice(i * OC, (i + 1) * OC)
        in_maps.append({
            "xt": xP,
            "wqt": _perm(np.ascontiguousarray(np.asarray(wq, np.float32)[sl, :].T), OC),
            "wkt": _perm(np.ascontiguousarray(np.asarray(wk, np.float32)[sl, :].T), OC),
            "wvt": _perm(np.ascontiguousarray(np.asarray(wv, np.float32)[sl, :].T), OC),
            "wot": woP,
        })
    return in_maps


def assemble(results):
    return np.concatenate(
        [results[i]["out"].astype(np.float32) for i in range(NCORES)], axis=0
    ).reshape(B, T, D)


def kernel(x, wq, wk, wv, wo):
    nc = get_nc()
    in_maps = make_in_maps(x, wq, wk, wv, wo)
    res = run_bass_kernel_spmd(nc, in_maps, list(range(NCORES)), trace=False)
    return assemble(res.results)


if __name__ == "__main__":
    rng = np.random.default_rng(0)
    s = 1.0 / math.sqrt(D)
    x = rng.standard_normal((B, T, D), dtype=np.float32)
    wq = (rng.standard_normal((D, D), dtype=np.float32) * s)
    wk = (rng.standard_normal((D, D), dtype=np.float32) * s)
    wv = (rng.standard_normal((D, D), dtype=np.float32) * s)
    wo = (rng.standard_normal((D, D), dtype=np.float32) * s)
    out = kernel(x, wq, wk, wv, wo)
    print("out", out.shape, out.dtype, np.abs(out).mean())


# revision 27
# speedup vs baseline: 1.0931x; 1.0931x over previous
"""Multi-head causal attention (B=2, T=2048, D=2048, H=16, dk=128) on 8 TRN2 NeuronCores.

Strategy (tensor-parallel over heads, 2 heads/core), bf16 matmul operands:
  - Host prep: all operands pre-permuted into the exact SBUF column layouts
    ([128, ...] partition-major) so every big device DMA is a contiguous 2D
    copy — descriptor issue is trivial and transfers run at full HBM rate.
    Everything cast to bfloat16 (PSUM accumulation stays fp32; rel tolerance
    is 2e-2, bf16 operands land ~6e-3).
  - Per core: QT/KT = PE matmuls producing Q^T/K^T layouts [dk, tokens];
    V natural [tokens, dk] via PE transposes. Wo.T (8 MB bf16) is fully
    SBUF-resident, prefetched during the projection phase.
  - A short burst of identity matmuls at t=0 keeps the PE HAM clock warm
    while the first weight/x transfers land.
  - Attention with TRANSPOSED scores: S^T[kk, q] chunks per kk-tile so the
    exp'd probabilities land directly in P^T layout (no PE transposes of P).
    No max-subtraction (scores are ~N(0,1); exp cannot overflow). Softmax
    denominator via ones[128,128] matmul over P^T partitions (result arrives
    pre-broadcast to all partitions); 1/d via DVE reciprocal_approx_fast read
    straight from PSUM. Diagonal kk-tiles stream only the live q-range
    (s_off) through the score, denominator AND context matmuls, so no zero
    fill of P^T is needed and the PE skips the masked triangle at 128-col
    granularity.
  - ctx^T [dk, tokens] per head -> AllToAll (1 MB/core) so each core gets all
    16 heads' ctx^T for its 512-token slice; cme/cmo gathered on the gpsimd
    queue immediately after each collective. Local Wo projection with
    even/odd head split: evens (ready after AllToAll#0) accumulate during
    the second collective, partials spill to SBUF; odds then accumulate in
    fresh PSUM in four o4-groups, each group's DVE merge + output store
    overlapping the next group's matmuls (spreads the 4 MB output DMA).
  - Host: concatenate the 8 row-slices.
  - Queue placement: xt chunk loads + collectives + cm gathers on gpsimd,
    weight loads + csb/out stores on sync, exps + PSUM spills on scalar.
"""

import math
import numpy as np
import ml_dtypes
from contextlib import ExitStack

import concourse.tile as tile
import concourse.mybir as mybir
from concourse import bacc
from concourse.bass_utils import run_bass_kernel_spmd

B, T, D = 2, 2048, 2048
H, DK = 16, 128
NCORES = 8
HL = H // NCORES            # 2 heads per core
OC = HL * DK                # 256 out dims per core
TT = B * T                  # 4096 flat tokens
TCHUNK = 512
NTC = TT // TCHUNK          # 8 token chunks (proj)
NKT = D // 128              # 16 contraction tiles
NQC = T // TCHUNK           # 4 q-chunks per batch
SCALE = 1.0 / math.sqrt(DK)
F32 = mybir.dt.float32
MDT = mybir.dt.bfloat16
BF16 = ml_dtypes.bfloat16
MASK_VAL = -1e30

_CACHE = {}


def build():
    nc = bacc.Bacc("TRN2", target_bir_lowering=False, debug=False, num_devices=NCORES)

    # host-pre-permuted layouts: partition-major, contiguous per DMA
    xt_d = nc.dram_tensor("xt", [128, NTC * NKT * TCHUNK], MDT, kind="ExternalInput")
    wqt_d = nc.dram_tensor("wqt", [128, NKT * OC], MDT, kind="ExternalInput")
    wkt_d = nc.dram_tensor("wkt", [128, NKT * OC], MDT, kind="ExternalInput")
    wvt_d = nc.dram_tensor("wvt", [128, NKT * OC], MDT, kind="ExternalInput")
    wot_d = nc.dram_tensor("wot", [128, NKT * D], MDT, kind="ExternalInput")
    out_d = nc.dram_tensor("out", [TT // NCORES, D], MDT, kind="ExternalOutput")

    with tile.TileContext(nc) as tc, ExitStack() as ctx:
        psum = ctx.enter_context(tc.tile_pool(name="ps", bufs=8, space="PSUM"))
        dram = ctx.enter_context(tc.tile_pool(name="dram", bufs=1, space="DRAM"))
        persist = ctx.enter_context(tc.tile_pool(name="persist", bufs=1))
        small = ctx.enter_context(tc.tile_pool(name="small", bufs=2))

        # ---- persistent SBUF: QT/KT [128, HL*TT] (cols: h*TT + flat_tok),
        # V [128, 32*256], full Wo.T [128, 16*2048] (cols: c16*D + o)
        QT = persist.tile([128, HL * TT], MDT, name="QTs")
        KT = persist.tile([128, HL * TT], MDT, name="KTs")
        Vs = persist.tile([128, (TT // 128) * OC], MDT, name="Vs")

        # ---- identity (needed by proj-phase V transposes + PE warmup)
        ident = persist.tile([128, 128], MDT, name="ident")
        with tc.tile_pool(name="cstage0", bufs=1) as cstage0:
            ident_f = cstage0.tile([128, 128], F32, name="ident_f")
            from concourse.masks import make_identity
            make_identity(nc, ident_f[:])
            nc.vector.tensor_copy(ident[:], ident_f[:])

        # ---- PE warmup: keep HAM at full clock while the first loads land
        warm = psum.tile([128, 128], F32, tag="mm", name="warm")
        for _ in range(40):
            nc.tensor.matmul(warm[:], ident[:], ident[:], start=True, stop=True)

        # =================== Phase 1: QKV projections ===================
        wope = ctx.enter_context(tc.tile_pool(name="wope", bufs=1))
        with tc.tile_pool(name="proj", bufs=1) as projp, \
             tc.tile_pool(name="xtp", bufs=2) as xtp:
            # lo/hi halves as separate tiles so the first matmuls depend only
            # on the first transfers
            HW2 = 8 * OC
            w_sbs = {}
            # one engine queue per weight tensor: three 0.75 MB transfers run
            # concurrently right after the preamble
            for kh in range(2):
                for wname, w_d, eng in (("q", wqt_d, nc.sync), ("k", wkt_d, nc.scalar),
                                        ("v", wvt_d, nc.sync if kh else nc.scalar)):
                    w_sb = projp.tile([128, HW2], MDT, name=f"w{wname}{kh}_sb")
                    w_sbs[(wname, kh)] = w_sb
                    eng.dma_start(out=w_sb[:], in_=w_d.ap()[:, kh * HW2:(kh + 1) * HW2])

            # full Wo.T prefetch (8 MB bf16) in quarters, interleaved on the
            # gpsimd queue late in proj so it never contends with the weight
            # and first-chunk loads; it streams long before phase 3 needs it.
            wo_sb = None
            WOQ = NKT * D // 4

            XW = NKT * TCHUNK
            for tcx in range(NTC):
                xts = []
                for kh in range(2):
                    xt = xtp.tile([128, XW // 2], MDT, tag=f"xt{kh}", name=f"xt_{tcx}_{kh}")
                    xts.append(xt)
                    nc.gpsimd.dma_start(
                        out=xt[:],
                        in_=xt_d.ap()[:, tcx * XW + kh * (XW // 2): tcx * XW + (kh + 1) * (XW // 2)],
                    )
                if tcx >= 3 and tcx % 2 == 1:
                    if wo_sb is None:
                        wo_sb = wope.tile([128, NKT * D], MDT, name="wo_sb")
                    wq4 = (tcx - 3) // 2
                    nc.gpsimd.dma_start(out=wo_sb[:, wq4 * WOQ:(wq4 + 1) * WOQ],
                                        in_=wot_d.ap()[:, wq4 * WOQ:(wq4 + 1) * WOQ])

                qp = [psum.tile([128, 512], F32, tag="mm", name=f"qp{tcx}_{o}") for o in range(HL)]
                kp = [psum.tile([128, 512], F32, tag="mm", name=f"kp{tcx}_{o}") for o in range(HL)]
                # V computed directly in natural [token, head*dk] layout:
                # stationary = x^T token-subtile, moving = Wv k-slice (N=256).
                # Two token-subtiles share one PSUM bank; only the first gets
                # start=True (bank-wide has_written clear), the second then
                # overwrites-where-clear.
                vn = [psum.tile([128, 512], F32, tag="mm", name=f"vn{tcx}_{i}") for i in range(2)]
                for k in range(NKT):
                    st, sp = (k == 0), (k == NKT - 1)
                    kh, kr = divmod(k, 8)
                    xk = xts[kh][:, kr * TCHUNK:(kr + 1) * TCHUNK]
                    for o in range(HL):
                        for w, ps in (("q", qp), ("k", kp)):
                            nc.tensor.matmul(
                                ps[o][:],
                                w_sbs[(w, kh)][:, kr * OC + o * 128: kr * OC + (o + 1) * 128],
                                xk, start=st, stop=sp)
                    wvk = w_sbs[("v", kh)][:, kr * OC:(kr + 1) * OC]
                    for t4 in range(4):
                        nc.tensor.matmul(
                            vn[t4 // 2][:, (t4 % 2) * 256:(t4 % 2) * 256 + 256],
                            xk[:, t4 * 128:(t4 + 1) * 128],
                            wvk,
                            start=(st and t4 % 2 == 0), stop=sp)
                for o in range(HL):
                    dst = slice(o * TT + tcx * TCHUNK, o * TT + (tcx + 1) * TCHUNK)
                    nc.scalar.copy(QT[:, dst], qp[o][:])
                    nc.scalar.copy(KT[:, dst], kp[o][:])
                for i in range(2):
                    t32 = tcx * 4 + 2 * i
                    nc.vector.tensor_copy(Vs[:, t32 * OC: (t32 + 2) * OC], vn[i][:])

        # ---- last Wo.T quarter (emitted after proj)
        nc.gpsimd.dma_start(out=wo_sb[:, 3 * WOQ:4 * WOQ],
                            in_=wot_d.ap()[:, 3 * WOQ:4 * WOQ])

        # ---- attention constants (emitted after proj so the first xt DMAs lead)
        maskT = persist.tile([128, 128], F32, name="maskT")
        nc.gpsimd.memset(maskT[:], 0.0)
        # keep 0 where q >= kk (predicate -x + y >= 0), else MASK_VAL
        nc.gpsimd.affine_select(
            out=maskT[:], in_=maskT[:], compare_op=mybir.AluOpType.is_ge,
            fill=MASK_VAL, base=0, pattern=[[1, 128]], channel_multiplier=-1,
        )
        onesk = persist.tile([128, 128], MDT, name="onesk")
        with tc.tile_pool(name="cstage", bufs=1) as cstage:
            ones_f = cstage.tile([128, 128], F32, name="ones_f")
            nc.gpsimd.memset(ones_f[:], 1.0)
            nc.vector.tensor_copy(onesk[:], ones_f[:])

        # =================== Phase 2: attention ===================
        a2a_in = [dram.tile([NCORES, 128, TCHUNK], MDT, name=f"a2a_in{h}") for h in range(HL)]
        a2a_out = [dram.tile([NCORES, 128, TCHUNK], MDT, name=f"a2a_out{h}")
                   for h in range(HL)]

        cm = [None, None]

        with tc.tile_pool(name="ptp", bufs=2) as ptp, \
             tc.tile_pool(name="rdp", bufs=3) as rdp:
            for hl in range(HL):
                for J in reversed(range(NQC)):
                    for b in range(B):
                        base = hl * TT + b * T
                        nkk = 4 * J + 4
                        ptiles = []

                        def emit_score(kk):
                            s_off = max(0, (kk - 4 * J) * 128)
                            npr = 512 - s_off
                            pt = ptp.tile([128, 512], MDT, tag=f"pt{kk}", name=f"p_{hl}{b}{J}_{kk}")
                            ptiles.append((pt, s_off))
                            st = psum.tile([128, 512], F32, tag="mm", name=f"st{hl}{b}{J}_{kk}")
                            nc.tensor.matmul(
                                st[:, :npr],
                                KT[:, base + kk * 128: base + (kk + 1) * 128],
                                QT[:, base + J * 512 + s_off: base + (J + 1) * 512],
                                start=True, stop=True,
                            )
                            if kk >= 4 * J:  # diagonal tile: causal mask
                                nc.vector.tensor_add(st[:, 0:128], st[:, 0:128], maskT[:])
                            nc.scalar.activation(pt[:, s_off:512], st[:, :npr],
                                                 mybir.ActivationFunctionType.Exp, scale=SCALE)

                        # software pipeline: keep the exp (ACT) two tiles ahead
                        # of the PE-side dp/cp consumers so neither engine stalls
                        dp = psum.tile([128, 512], F32, tag="mm", name=f"dp{hl}{b}{J}")
                        cp = psum.tile([128, 512], F32, tag="mm", name=f"cp{hl}{b}{J}")
                        emit_score(0)
                        emit_score(1)
                        for kk in range(nkk):
                            if kk + 2 < nkk:
                                emit_score(kk + 2)
                            pt, s_off = ptiles[kk]
                            nc.tensor.matmul(dp[:, s_off:512], onesk[:], pt[:, s_off:512],
                                             start=(kk == 0), stop=(kk == nkk - 1))
                            nc.tensor.matmul(
                                cp[:, s_off:512],
                                Vs[:, (b * 16 + kk) * OC + hl * 128: (b * 16 + kk) * OC + (hl + 1) * 128],
                                pt[:, s_off:512],
                                start=(kk == 0), stop=(kk == nkk - 1),
                            )
                        rd = rdp.tile([128, 512], F32, tag="rd", name=f"rd_{hl}{b}{J}")
                        nc.vector.reciprocal_approx_fast(rd[:], dp[:])
                        csb = rdp.tile([128, 512], MDT, tag="csb", name=f"csb{hl}{b}{J}")
                        nc.vector.tensor_mul(csb[:], cp[:], rd[:])
                        nc.sync.dma_start(out=a2a_in[hl][b * NQC + J], in_=csb[:])
                nc.gpsimd.collective_compute(
                    "AllToAll", mybir.AluOpType.bypass,
                    replica_groups=[list(range(NCORES))],
                    ins=[a2a_in[hl].opt()], outs=[a2a_out[hl].opt()],
                )
                # gather all 16 heads' ctx^T for my 512-token slice right away
                cm[hl] = wope.tile([128, NCORES * TCHUNK], MDT, name=f"cm{hl}")
                nc.gpsimd.dma_start(
                    out=cm[hl][:].rearrange("p (c t) -> p c t", c=NCORES),
                    in_=a2a_out[hl].rearrange("c p t -> p c t"),
                )

        # =================== Phase 3: output projection ===================
        # Evens (= heads from AllToAll#0) accumulate for all o4 groups while
        # AllToAll#1 is in flight, spilling partials to SBUF; odds then run
        # in four o4-groups so each group's merge+store overlaps the next
        # group's matmuls.
        with tc.tile_pool(name="accp", bufs=1) as accp, \
             tc.tile_pool(name="outp", bufs=3) as outp:
            acc = {}
            for pi, (oa, ob) in enumerate(((0, 1), (2, 3))):
                ops = {o4: [psum.tile([128, 512], F32, tag="mm", name=f"ope{o4}_{t}")
                            for t in range(4)]
                       for o4 in (oa, ob)}
                for ci in range(NCORES):
                    i = ci
                    for o4 in (oa, ob):
                        for t4 in range(4):
                            nc.tensor.matmul(
                                ops[o4][t4][:],
                                cm[0][:, i * 512 + t4 * 128: i * 512 + (t4 + 1) * 128],
                                wo_sb[:, 2 * i * D + o4 * 512: 2 * i * D + (o4 + 1) * 512],
                                start=(ci == 0), stop=(ci == NCORES - 1))
                for o4 in (oa, ob):
                    for t4 in range(4):
                        a_ = accp.tile([128, 512], F32, name=f"acc{o4}_{t4}")
                        nc.scalar.copy(a_[:], ops[o4][t4][:])
                        acc[(o4, t4)] = a_
            for o4 in range(4):
                for t2 in range(2):
                    ops = [psum.tile([128, 512], F32, tag="mm", name=f"opo{o4}_{t2}_{t}")
                           for t in range(2)]
                    for ci in range(NCORES):
                        i = ci
                        for tt in range(2):
                            t4 = t2 * 2 + tt
                            nc.tensor.matmul(
                                ops[tt][:],
                                cm[1][:, i * 512 + t4 * 128: i * 512 + (t4 + 1) * 128],
                                wo_sb[:, (2 * i + 1) * D + o4 * 512: (2 * i + 1) * D + (o4 + 1) * 512],
                                start=(ci == 0), stop=(ci == NCORES - 1))
                    for tt in range(2):
                        t4 = t2 * 2 + tt
                        ot = outp.tile([128, 512], MDT, tag="ot", name=f"ot{o4}_{t4}")
                        nc.vector.tensor_add(ot[:], ops[tt][:], acc[(o4, t4)][:])
                        nc.sync.dma_start(
                            out=out_d.ap()[t4 * 128:(t4 + 1) * 128, o4 * 512:(o4 + 1) * 512],
                            in_=ot[:],
                        )

    nc.compile()
    return nc


def get_nc():
    if "nc" not in _CACHE:
        _CACHE["nc"] = build()
    return _CACHE["nc"]


def _perm(wT, inner):
    """[NKT*128, inner] row-major -> [128, NKT*inner] partition-major bf16."""
    return np.ascontiguousarray(
        wT.reshape(NKT, 128, inner).transpose(1, 0, 2).reshape(128, NKT * inner)
    ).astype(BF16)


def make_in_maps(x, wq, wk, wv, wo):
    x = np.asarray(x, dtype=np.float32)
    xT = np.ascontiguousarray(x.reshape(TT, D).T)          # [D, TT]
    # [128, tcx, kt, t] layout: per token-chunk contiguous
    xP = np.ascontiguousarray(
        xT.reshape(NKT, 128, NTC, TCHUNK).transpose(1, 2, 0, 3).reshape(128, -1)
    ).astype(BF16)
    woP = _perm(np.ascontiguousarray(np.asarray(wo, np.float32).T), D)
    in_maps = []
    for i in range(NCORES):
        sl = slice(i * OC, (i + 1) * OC)
        in_maps.append({
            "xt": xP,
            "wqt": _perm(np.ascontiguousarray(np.asarray(wq, np.float32)[sl, :].T), OC),
            "wkt": _perm(np.ascontiguousarray(np.asarray(wk, np.float32)[sl, :].T), OC),
            "wvt": _perm(np.ascontiguousarray(np.asarray(wv, np.float32)[sl, :].T), OC),
            "wot": woP,
        })
    return in_maps


def assemble(results):
    return np.concatenate(
        [results[i]["out"].astype(np.float32) for i in range(NCORES)], axis=0
    ).reshape(B, T, D)


def kernel(x, wq, wk, wv, wo):
    nc = get_nc()
    in_maps = make_in_maps(x, wq, wk, wv, wo)
    res = run_bass_kernel_spmd(nc, in_maps, list(range(NCORES)), trace=False)
    return assemble(res.results)


if __name__ == "__main__":
    rng = np.random.default_rng(0)
    s = 1.0 / math.sqrt(D)
    x = rng.standard_normal((B, T, D), dtype=np.float32)
    wq = (rng.standard_normal((D, D), dtype=np.float32) * s)
    wk = (rng.standard_normal((D, D), dtype=np.float32) * s)
    wv = (rng.standard_normal((D, D), dtype=np.float32) * s)
    wo = (rng.standard_normal((D, D), dtype=np.float32) * s)
    out = kernel(x, wq, wk, wv, wo)
    print("out", out.shape, out.dtype, np.abs(out).mean())
